# revision 6
# baseline (speedup 1.0000x reference)
"""BitNet b1.58 transformer block on 8 Trainium2 NeuronCores (Bass/Tile).

Sharding (tensor-parallel per the spec hint):
  - attention: 4 q-heads + 1 kv-head per core (GQA group-aligned), sequence-
    sharded LayerNorm+quant with AllGather of the quantized activations;
    AllToAll redistributes attention output to token-sharding so o_proj
    needs no absmax collective and no big reduce.
  - FFN: fc1 rows (1024 per core) column-parallel; fc2 partials pre-scaled
    by the per-token dequant and ReduceScattered in bf16 back to token
    shards, chunked even/odd token tiles so collectives overlap compute.

Numerics: BitNet quantization makes matmuls integer-exact — activations are
round()ed to [-127,127] integers carried in bf16 (exact), ternary weights
are pre-quantized on host (exact in bf16), PSUM accumulates in fp32 (sums
bounded < 2^24, exact). Dequant scales are folded into per-partition
ACT/DVE scale operands. round() is the fp32 magic-number trick (exact RNE,
matches jnp.round).
"""
import os
import numpy as np
import ml_dtypes

import concourse.bass as bass
import concourse.mybir as mybir
import concourse.tile as tile
from concourse import bacc
from concourse import bass_utils
from concourse.tile_rust import add_dep_helper

F32 = mybir.dt.float32
BF16 = mybir.dt.bfloat16
AL = mybir.AluOpType
AF = mybir.ActivationFunctionType
AX = mybir.AxisListType

NCORES = 8
D = 2048
T = 2048
TLOC = T // NCORES          # 256 tokens per core
NTT = T // 128              # 16 token tiles
HD = 64
QD = 256                    # q out-dims per core (4 heads x 64)
FLOC = 1024                 # fc1 rows per core
EPS = 1e-6
RC = 12582912.0             # 1.5 * 2**23 -> fp32 RNE round trick
INV127 = float(np.float32(1.0) / np.float32(127.0))
RG = [list(range(NCORES))]

_CACHE = {}


def _build(affine1: bool, affine2: bool):
    nc = bacc.Bacc("TRN2", target_bir_lowering=False, debug=False,
                   enable_asserts=False, num_devices=NCORES)

    x_io = nc.dram_tensor("x", [TLOC, D], F32, kind="ExternalInput").ap()
    wqT = nc.dram_tensor("wqT", [D, QD], BF16, kind="ExternalInput").ap()
    wkT = nc.dram_tensor("wkT", [D, 128], BF16, kind="ExternalInput").ap()
    wvT = nc.dram_tensor("wvT", [D, HD], BF16, kind="ExternalInput").ap()
    woT = nc.dram_tensor("woT", [D, D], BF16, kind="ExternalInput").ap()
    w1T = nc.dram_tensor("w1T", [D, FLOC], BF16, kind="ExternalInput").ap()
    w2T = nc.dram_tensor("w2T", [FLOC, D], BF16, kind="ExternalInput").ap()
    cmask_io = nc.dram_tensor("cmask", [128, 128], BF16, kind="ExternalInput").ap()
    consts = nc.dram_tensor("consts", [1, 8], F32, kind="ExternalInput").ap()
    affs = {}
    if affine1:
        affs['g1'] = nc.dram_tensor("g1", [1, D], F32, kind="ExternalInput").ap()
        affs['b1'] = nc.dram_tensor("b1", [1, D], F32, kind="ExternalInput").ap()
    if affine2:
        affs['g2'] = nc.dram_tensor("g2", [1, D], F32, kind="ExternalInput").ap()
        affs['b2'] = nc.dram_tensor("b2", [1, D], F32, kind="ExternalInput").ap()
    out_io = nc.dram_tensor("out", [TLOC, D], F32, kind="ExternalOutput").ap()

    def brow(dram_ap, n):
        """[1, n] DRAM row broadcast onto 128 partitions."""
        return bass.AP(tensor=dram_ap.tensor, offset=dram_ap.offset,
                       ap=[[0, 128], [1, n]])

    def tap(tens, offset, dims):
        return bass.AP(tensor=tens.tensor, offset=tens.offset + offset, ap=dims)

    with tile.TileContext(nc) as tc:
        from contextlib import ExitStack
        with ExitStack() as top:
            dram = top.enter_context(tc.tile_pool(name="dram", bufs=1, space="DRAM"))
            constp = top.enter_context(tc.tile_pool(name="constp", bufs=1))
            small = top.enter_context(tc.tile_pool(name="small", bufs=8))
            x1p = top.enter_context(tc.tile_pool(name="x1p", bufs=1))

            # ----------------- DRAM scratch -----------------
            ag1_in = dram.tile([TLOC, D], BF16)
            ag1_out = dram.tile([T, D], BF16, addr_space="Shared")
            s1_in = dram.tile([1, TLOC], F32)
            s1_out = dram.tile([1, T], F32, addr_space="Shared")
            a2a_in = dram.tile([T, QD], F32)
            a2a_out = dram.tile([T, QD], F32)
            oq_dram = dram.tile([TLOC, D], BF16)
            ag2_in = dram.tile([TLOC, D], BF16)
            ag2_out = dram.tile([T, D], BF16, addr_space="Shared")
            s2_in = dram.tile([1, TLOC], F32)
            s2_out = dram.tile([1, T], F32, addr_space="Shared")
            g_dram = dram.tile([T, FLOC], F32)
            fq_dram = [dram.tile([T // 2, FLOC], BF16, tag=f"fqd{s}",
                                 name=f"fqd{s}") for s in range(2)]
            am_in = [dram.tile([1, T // 2], F32, tag=f"ami{s}", name=f"ami{s}")
                     for s in range(2)]
            am_out = [dram.tile([1, T // 2], F32, addr_space="Shared",
                                tag=f"amo{s}", name=f"amo{s}") for s in range(2)]
            rs_in = [dram.tile([T // 2, D], BF16, tag=f"rsi{s}", name=f"rsi{s}")
                     for s in range(2)]
            rs_out = [dram.tile([128, D], BF16, tag=f"rso{s}", name=f"rso{s}")
                      for s in range(2)]

            # ----------------- constants -----------------
            cb = constp.tile([128, 8], F32)
            nc.gpsimd.dma_start(cb, brow(consts, 8))
            cb_cq, cb_mv, cb_mo, cb_m1, cb_m2 = (cb[:, i:i + 1] for i in range(5))
            cmask = constp.tile([128, 128], BF16)
            nc.sync.dma_start(cmask, cmask_io)
            zcol = constp.tile([128, 1], F32)
            nc.vector.memset(zcol, 0.0)
            ecol = constp.tile([128, 1], F32)
            nc.vector.memset(ecol, EPS)
            gb_sb = {}
            for k, ap_ in affs.items():
                t = constp.tile([128, D], F32, tag=f"aff_{k}", name=f"aff_{k}")
                nc.gpsimd.dma_start(t, brow(ap_, D))
                gb_sb[k] = t

            x1 = x1p.tile([128, 2, D], F32)

            # ========== LayerNorm + absmax int8-quant helper ==========
            def ln_quant(lnp, src, affine, gk, bk, aq_dst, sr_dst):
                stats = small.tile([128, 4, 6], F32, tag="stats")
                for sg in range(4):
                    nc.vector.bn_stats(stats[:, sg, :], src[:, 512 * sg:512 * (sg + 1)])
                mv = small.tile([128, 2], F32, tag="mv")
                nc.vector.bn_aggr(mv, stats)
                mean, var = mv[:, 0:1], mv[:, 1:2]
                std = small.tile([128, 1], F32, tag="std")
                nc.scalar.activation(std, var, AF.Sqrt, bias=ecol)
                rstd = small.tile([128, 1], F32, tag="rstd")
                nc.vector.reciprocal(rstd, std)
                amh = small.tile([128, 1], F32, tag="amh")
                h = None
                if not affine:
                    xm = lnp.tile([128, D], F32, tag="lntmp", bufs=2)
                    nc.vector.tensor_scalar(xm, src, mean, None, AL.subtract)
                    am = small.tile([128, 1], F32, tag="am")
                    nc.vector.tensor_reduce(am, xm, axis=AX.X, op=AL.max,
                                            apply_absolute_value=True)
                    nc.vector.tensor_tensor(amh, am, rstd, op=AL.mult)
                else:
                    nb0 = small.tile([128, 1], F32, tag="nb0")
                    nc.vector.tensor_tensor(nb0, mean, rstd, op=AL.mult)
                    nc.vector.tensor_scalar_mul(nb0, nb0, -1.0)
                    h = lnp.tile([128, D], F32, tag="lntmp", bufs=2)
                    nc.scalar.activation(h, src, AF.Identity, bias=nb0, scale=rstd)
                    nc.vector.tensor_tensor(h, h, gb_sb[gk], op=AL.mult)
                    nc.vector.tensor_tensor(h, h, gb_sb[bk], op=AL.add)
                    nc.vector.tensor_reduce(amh, h, axis=AX.X, op=AL.max,
                                            apply_absolute_value=True)
                amc = small.tile([128, 1], F32, tag="amc")
                nc.vector.tensor_scalar_max(amc, amh, 1e-5)
                sr = small.tile([128, 1], F32, tag="sr")
                nc.vector.tensor_scalar_mul(sr, amc, INV127)
                rec = small.tile([128, 1], F32, tag="rec")
                nc.vector.reciprocal(rec, amc)
                s = small.tile([128, 1], F32, tag="s")
                nc.vector.tensor_scalar_mul(s, rec, 127.0)
                y = lnp.tile([128, D], F32, tag="lny", bufs=2)
                if not affine:
                    sc = small.tile([128, 1], F32, tag="sc")
                    nc.vector.tensor_tensor(sc, rstd, s, op=AL.mult)
                    nb = small.tile([128, 1], F32, tag="nb")
                    nc.vector.tensor_tensor(nb, mean, sc, op=AL.mult)
                    nc.vector.tensor_scalar_mul(nb, nb, -1.0)
                    nc.scalar.activation(y, src, AF.Identity, bias=nb, scale=sc)
                else:
                    nc.scalar.activation(y, h, AF.Copy, bias=0.0, scale=s)
                aq = lnp.tile([128, D], BF16, tag="lnq", bufs=2)
                nc.vector.tensor_scalar(aq, y, RC, -RC, AL.add, AL.add)
                nc.sync.dma_start(aq_dst, aq)
                nc.sync.dma_start(sr_dst, sr)

            # =============== phase 1: LN1 + quant + AllGather ===============
            with tc.tile_pool(name="xcp", bufs=1) as xcp:
                xc = xcp.tile([128, 2, D], F32)
                for tt in range(2):
                    nc.sync.dma_start(xc[:, tt, :], x_io[128 * tt:128 * tt + 128, :])
                with tc.tile_pool(name="ln1", bufs=1) as ln1p:
                    for tt in range(2):
                        ln_quant(ln1p, xc[:, tt, :], affine1, 'g1', 'b1',
                                 ag1_in[128 * tt:128 * tt + 128, :],
                                 s1_in[0:1, 128 * tt:128 * tt + 128])
                nc.gpsimd.collective_compute(
                    "AllGather", AL.bypass, replica_groups=RG,
                    ins=[ag1_in.opt()], outs=[ag1_out.opt()])
                nc.gpsimd.collective_compute(
                    "AllGather", AL.bypass, replica_groups=RG,
                    ins=[s1_in.opt()], outs=[s1_out.opt()])

                s1col = constp.tile([128, 16], F32)
                nc.gpsimd.dma_start(s1col, tap(s1_out, 0, [[1, 128], [128, 16]]))
                e_sc = constp.tile([128, 16], F32)
                nc.vector.tensor_scalar(e_sc, s1col, cb_cq, None, AL.mult)
                v_sc = constp.tile([128, 16], F32)
                nc.vector.tensor_scalar(v_sc, s1col, cb_mv, None, AL.mult)

                # =============== phases 2-4: QKV + attention ===============
                with tc.tile_pool(name="qkvp", bufs=1) as qkvp:
                    qT = qkvp.tile([128, 2, T], BF16)
                    kT = qkvp.tile([128, T], BF16)
                    vt = qkvp.tile([128, NTT, HD + 1], BF16)
                    nc.vector.memset(vt[:, :, HD:HD + 1], 1.0)

                    with tc.tile_pool(name="aTp", bufs=1) as aTp:
                        a1qT = aTp.tile([128, NTT, T], BF16)
                        for dt in range(NTT):
                            nc.sync.dma_start_transpose(
                                a1qT[:, dt, :], ag1_out[:, 128 * dt:128 * dt + 128])
                        s1row = aTp.tile([128, T], F32)
                        nc.gpsimd.dma_start(s1row, tap(s1_out, 0, [[0, 128], [1, T]]))
                        wq_sb = aTp.tile([128, NTT, QD], BF16)
                        nc.sync.dma_start(
                            wq_sb, tap(wqT, 0, [[QD, 128], [QD * 128, NTT], [1, QD]]))
                        wk_sb = aTp.tile([128, NTT, 128], BF16)
                        nc.sync.dma_start(
                            wk_sb, tap(wkT, 0, [[128, 128], [128 * 128, NTT], [1, 128]]))
                        wv_sb = aTp.tile([128, NTT, HD], BF16)
                        nc.sync.dma_start(
                            wv_sb, tap(wvT, 0, [[HD, 128], [HD * 128, NTT], [1, HD]]))

                        with tc.tile_pool(name="ps3", bufs=1, space="PSUM") as ps3:
                            for qt in range(2):
                                for ch in range(4):
                                    ps = ps3.tile([128, 512], F32, tag="qk", bufs=4,
                                                  name=f"psq{qt}{ch}")
                                    for dt in range(NTT):
                                        nc.tensor.matmul(
                                            ps, wq_sb[:, dt, 128 * qt:128 * qt + 128],
                                            a1qT[:, dt, 512 * ch:512 * ch + 512],
                                            start=(dt == 0), stop=(dt == NTT - 1))
                                    nc.vector.tensor_tensor(
                                        qT[:, qt, 512 * ch:512 * ch + 512], ps,
                                        s1row[:, 512 * ch:512 * ch + 512], op=AL.mult)
                            for ch in range(4):
                                ps = ps3.tile([128, 512], F32, tag="qk", bufs=4,
                                              name=f"psk{ch}")
                                for dt in range(NTT):
                                    nc.tensor.matmul(
                                        ps, wk_sb[:, dt, :],
                                        a1qT[:, dt, 512 * ch:512 * ch + 512],
                                        start=(dt == 0), stop=(dt == NTT - 1))
                                nc.vector.tensor_copy(kT[:, 512 * ch:512 * ch + 512], ps)
                            for j in range(NTT):
                                ps = ps3.tile([128, HD], F32, tag="pv", bufs=4,
                                              name=f"psv{j}")
                                for dt in range(NTT):
                                    nc.tensor.matmul(
                                        ps, a1qT[:, dt, 128 * j:128 * j + 128],
                                        wv_sb[:, dt, :],
                                        start=(dt == 0), stop=(dt == NTT - 1))
                                nc.scalar.activation(vt[:, j, 0:HD], ps, AF.Copy,
                                                     bias=0.0, scale=v_sc[:, j:j + 1])

                    # ---------------- attention ----------------
                    with tc.tile_pool(name="ps4", bufs=1, space="PSUM") as ps4, \
                         tc.tile_pool(name="pp", bufs=6) as pp:
                        for qg in range(4):
                            opsum = [ps4.tile([128, 4 * (HD + 1)], F32,
                                              tag=f"ob{qi}", bufs=1,
                                              name=f"ops{qg}{qi}")
                                     for qi in range(4)]
                            first_mm = [None] * 4
                            seen = [[False] * 4 for _ in range(4)]
                            for j in range(4 * qg + 4):
                                for h in range(4):
                                    scp = ps4.tile([128, 512], F32, tag="sc",
                                                   bufs=3, name=f"scp{qg}{j}{h}")
                                    nc.tensor.matmul(
                                        scp,
                                        kT[64 * (h % 2):64 * (h % 2) + 64,
                                           128 * j:128 * j + 128],
                                        qT[64 * (h % 2):64 * (h % 2) + 64, h // 2,
                                           512 * qg:512 * qg + 512],
                                        start=True, stop=True)
                                    p_sb = pp.tile([128, 512], BF16, tag="p",
                                                   name=f"p{qg}{j}{h}")
                                    nc.scalar.activation(p_sb, scp, AF.Exp, bias=zcol,
                                                         scale=e_sc[:, j:j + 1])
                                    m = j - 4 * qg
                                    if m >= 0:
                                        nc.vector.tensor_tensor(
                                            p_sb[:, 128 * m:128 * m + 128],
                                            p_sb[:, 128 * m:128 * m + 128],
                                            cmask, op=AL.mult)
                                    for qi in range(4):
                                        if j > 4 * qg + qi:
                                            continue
                                        isf = (j == 0 and h == 0)
                                        mm = nc.tensor.matmul(
                                            opsum[qi][:, (HD + 1) * h:(HD + 1) * (h + 1)],
                                            p_sb[:, 128 * qi:128 * qi + 128],
                                            vt[:, j, :],
                                            start=isf,
                                            stop=(j == 4 * qg + qi and h == 3),
                                            skip_group_check=True)
                                        if isf:
                                            first_mm[qi] = mm
                                        elif not seen[qi][h]:
                                            add_dep_helper(mm.ins, first_mm[qi].ins,
                                                           sync=False,
                                                           reason="bank clear order")
                                        seen[qi][h] = True
                                qi = j - 4 * qg
                                if qi >= 0:
                                    ot = pp.tile([128, QD], F32, tag="otrue",
                                                 bufs=4, name=f"ot{qg}{qi}")
                                    for h in range(4):
                                        orec = small.tile([128, 1], F32, tag="orec")
                                        nc.vector.reciprocal(
                                            orec,
                                            opsum[qi][:, (HD + 1) * h + HD:
                                                      (HD + 1) * h + HD + 1])
                                        nc.vector.tensor_scalar(
                                            ot[:, HD * h:HD * h + HD],
                                            opsum[qi][:, (HD + 1) * h:(HD + 1) * h + HD],
                                            orec, None, AL.mult)
                                    qtile = 4 * qg + qi
                                    nc.sync.dma_start(
                                        a2a_in[128 * qtile:128 * qtile + 128, :], ot)

                nc.gpsimd.collective_compute(
                    "AllToAll", AL.bypass, replica_groups=RG,
                    ins=[a2a_in.opt()], outs=[a2a_out.opt()])

                # =============== phase 5: o_proj + residual ===============
                with tc.tile_pool(name="op5", bufs=1) as op5, \
                     tc.tile_pool(name="ps5", bufs=1, space="PSUM") as ps5p:
                    oqT = op5.tile([128, NTT, TLOC], BF16)
                    osc = [None, None]
                    for tt in range(2):
                        ofull = op5.tile([128, D], F32, tag="ofull", bufs=2,
                                         name=f"ofull{tt}")
                        nc.sync.dma_start(
                            ofull, tap(a2a_out, 128 * tt * QD,
                                       [[QD, 128], [QD * TLOC, 8], [1, QD]]))
                        am = small.tile([128, 1], F32, tag="am5")
                        nc.vector.tensor_reduce(am, ofull, axis=AX.X, op=AL.max,
                                                apply_absolute_value=True)
                        amc = small.tile([128, 1], F32, tag="amc5")
                        nc.vector.tensor_scalar_max(amc, am, 1e-5)
                        rec = small.tile([128, 1], F32, tag="rec5")
                        nc.vector.reciprocal(rec, amc)
                        so = small.tile([128, 1], F32, tag="so5")
                        nc.vector.tensor_scalar_mul(so, rec, 127.0)
                        od = small.tile([128, 1], F32, tag=f"od5_{tt}",
                                        name=f"od5_{tt}", bufs=1)
                        nc.vector.tensor_scalar(od, amc, INV127, None, AL.mult)
                        nc.vector.tensor_scalar(od, od, cb_mo, None, AL.mult)
                        osc[tt] = od
                        oy = op5.tile([128, D], F32, tag="oy", bufs=2,
                                      name=f"oy{tt}")
                        nc.scalar.activation(oy, ofull, AF.Copy, bias=0.0, scale=so)
                        oq = op5.tile([128, D], BF16, tag="oqq", bufs=2,
                                      name=f"oqq{tt}")
                        nc.vector.tensor_scalar(oq, oy, RC, -RC, AL.add, AL.add)
                        nc.sync.dma_start(oq_dram[128 * tt:128 * tt + 128, :], oq)
                    for g in range(NTT):
                        nc.sync.dma_start_transpose(
                            oqT[:, g, :], oq_dram[:, 128 * g:128 * g + 128])
                    ps5 = [ps5p.tile([128, 512], F32, tag=f"p5{i}", bufs=1,
                                     name=f"ps5{i}") for i in range(8)]
                    for g in range(NTT):
                        wo_t = op5.tile([128, D], BF16, tag="wo", bufs=3,
                                        name=f"wo{g}")
                        nc.sync.dma_start(wo_t, woT[128 * g:128 * g + 128, :])
                        for tt in range(2):
                            for ch in range(4):
                                nc.tensor.matmul(
                                    ps5[tt * 4 + ch],
                                    oqT[:, g, 128 * tt:128 * tt + 128],
                                    wo_t[:, 512 * ch:512 * ch + 512],
                                    start=(g == 0), stop=(g == NTT - 1))
                    for tt in range(2):
                        for ch in range(4):
                            nc.vector.scalar_tensor_tensor(
                                x1[:, tt, 512 * ch:512 * ch + 512],
                                ps5[tt * 4 + ch], osc[tt],
                                xc[:, tt, 512 * ch:512 * ch + 512],
                                op0=AL.mult, op1=AL.add)

            # =============== phase 6: LN2 + quant + AllGather ===============
            with tc.tile_pool(name="ln2", bufs=1) as ln2p:
                for tt in range(2):
                    ln_quant(ln2p, x1[:, tt, :], affine2, 'g2', 'b2',
                             ag2_in[128 * tt:128 * tt + 128, :],
                             s2_in[0:1, 128 * tt:128 * tt + 128])
            nc.gpsimd.collective_compute(
                "AllGather", AL.bypass, replica_groups=RG,
                ins=[ag2_in.opt()], outs=[ag2_out.opt()])
            nc.gpsimd.collective_compute(
                "AllGather", AL.bypass, replica_groups=RG,
                ins=[s2_in.opt()], outs=[s2_out.opt()])

            # =============== phase 7: FFN ===============
            evens = list(range(0, NTT, 2))
            odds = list(range(1, NTT, 2))
            s2col = constp.tile([128, 16], F32)
            nc.gpsimd.dma_start(s2col, tap(s2_out, 0, [[1, 128], [128, 16]]))
            f_sc = constp.tile([128, 16], F32)
            nc.vector.tensor_scalar(f_sc, s2col, cb_m1, None, AL.mult)

            with tc.tile_pool(name="gpool", bufs=1) as gpool, \
                 tc.tile_pool(name="fqtp", bufs=1) as fqtp:
                sfr_t = {}
                with tc.tile_pool(name="fc1p", bufs=1) as fc1p:
                    a2qT = fc1p.tile([128, NTT, T], BF16)
                    for dt in range(NTT):
                        nc.sync.dma_start_transpose(
                            a2qT[:, dt, :], ag2_out[:, 128 * dt:128 * dt + 128])
                    w1_sb = fc1p.tile([128, NTT, FLOC], BF16)
                    nc.sync.dma_start(
                        w1_sb, tap(w1T, 0, [[FLOC, 128], [FLOC * 128, NTT], [1, FLOC]]))
                    with tc.tile_pool(name="ps7", bufs=1, space="PSUM") as ps7:
                        for s, tset in ((0, evens), (1, odds)):
                            for p, tt in enumerate(tset):
                                g_t = gpool.tile([128, FLOC], F32, tag="g", bufs=2,
                                                 name=f"g{tt}")
                                for fch in range(2):
                                    ps = ps7.tile([128, 512], F32, tag="f1", bufs=6,
                                                  name=f"psf{tt}{fch}")
                                    for dt in range(NTT):
                                        nc.tensor.matmul(
                                            ps, a2qT[:, dt, 128 * tt:128 * tt + 128],
                                            w1_sb[:, dt, 512 * fch:512 * fch + 512],
                                            start=(dt == 0), stop=(dt == NTT - 1))
                                    nc.vector.tensor_scalar(
                                        g_t[:, 512 * fch:512 * fch + 512], ps,
                                        f_sc[:, tt:tt + 1], 0.0, AL.mult, AL.max)
                                amg = small.tile([128, 1], F32, tag="amg")
                                nc.vector.tensor_reduce(amg, g_t, axis=AX.X, op=AL.max)
                                amf = small.tile([128, 1], F32, tag="amf")
                                nc.vector.tensor_tensor(amf, amg, amg, op=AL.mult)
                                nc.sync.dma_start(
                                    am_in[s][0:1, 128 * p:128 * p + 128], amf)
                                nc.sync.dma_start(
                                    g_dram[128 * tt:128 * tt + 128, :], g_t)
                            nc.gpsimd.collective_compute(
                                "AllReduce", AL.max, replica_groups=RG,
                                ins=[am_in[s].opt()], outs=[am_out[s].opt()])

                # quantize f, transpose, fc2, pre-scaled reduce-scatter
                with tc.tile_pool(name="fc2p", bufs=1) as fc2p:
                    w2_sb = fc2p.tile([128, 8, D], BF16)
                    nc.sync.dma_start(
                        w2_sb, tap(w2T, 0, [[D, 128], [D * 128, 8], [1, D]]))
                    for s, tset in ((0, evens), (1, odds)):
                        for p, tt in enumerate(tset):
                            amf = small.tile([128, 1], F32, tag="amf2")
                            nc.sync.dma_start(
                                amf, tap(am_out[s], 128 * p, [[1, 128], [0, 1]]))
                            amc = small.tile([128, 1], F32, tag="amc7")
                            nc.vector.tensor_scalar_max(amc, amf, 1e-5)
                            rec = small.tile([128, 1], F32, tag="rec7")
                            nc.vector.reciprocal(rec, amc)
                            sf = small.tile([128, 1], F32, tag="sf7")
                            nc.vector.tensor_scalar_mul(sf, rec, 127.0)
                            ssf = small.tile([128, 1], F32, tag="ssf7")
                            nc.scalar.activation(ssf, sf, AF.Sqrt, bias=zcol)
                            sfr = small.tile([128, 1], F32, tag=f"sfr{s}_{p}",
                                             name=f"sfr{s}_{p}", bufs=1)
                            nc.vector.tensor_scalar(sfr, amc, INV127, None, AL.mult)
                            nc.vector.tensor_scalar(sfr, sfr, cb_m2, None, AL.mult)
                            sfr_t[(s, p)] = sfr
                            gb = gpool.tile([128, FLOC], F32, tag="gb", bufs=2,
                                            name=f"gb{tt}")
                            nc.sync.dma_start(gb, g_dram[128 * tt:128 * tt + 128, :])
                            fy = gpool.tile([128, FLOC], F32, tag="fy", bufs=2,
                                            name=f"fy{tt}")
                            nc.scalar.activation(fy, gb, AF.Square, bias=zcol,
                                                 scale=ssf)
                            fq = gpool.tile([128, FLOC], BF16, tag="fq", bufs=2,
                                            name=f"fqt{tt}")
                            nc.vector.tensor_scalar(fq, fy, RC, -RC, AL.add, AL.add)
                            nc.sync.dma_start(
                                fq_dram[s][128 * p:128 * p + 128, :], fq)
                        fqT = fqtp.tile([128, 8, T // 2], BF16, tag="fqT", bufs=2,
                                        name=f"fqT{s}")
                        for ft in range(8):
                            nc.sync.dma_start_transpose(
                                fqT[:, ft, :], fq_dram[s][:, 128 * ft:128 * ft + 128])
                        with tc.tile_pool(name=f"ps8_{s}", bufs=1,
                                          space="PSUM") as ps8p:
                            for pr in range(4):
                                ps8 = [ps8p.tile([128, 512], F32, tag=f"p8{i}",
                                                 bufs=1, name=f"ps8_{s}{pr}{i}")
                                       for i in range(8)]
                                for ft in range(8):
                                    for ti in range(2):
                                        pos = 2 * pr + ti
                                        for ch in range(4):
                                            nc.tensor.matmul(
                                                ps8[ti * 4 + ch],
                                                fqT[:, ft, 128 * pos:128 * pos + 128],
                                                w2_sb[:, ft, 512 * ch:512 * ch + 512],
                                                start=(ft == 0), stop=(ft == 7))
                                for ti in range(2):
                                    pos = 2 * pr + ti
                                    st = fc2p.tile([128, D], BF16, tag="rsst",
                                                   bufs=3, name=f"rsst{s}{pos}")
                                    for ch in range(4):
                                        nc.vector.tensor_scalar(
                                            st[:, 512 * ch:512 * ch + 512],
                                            ps8[ti * 4 + ch], sfr_t[(s, pos)],
                                            None, AL.mult)
                                    nc.sync.dma_start(
                                        rs_in[s][128 * pos:128 * pos + 128, :], st)
                        nc.gpsimd.collective_compute(
                            "ReduceScatter", AL.add, replica_groups=RG,
                            ins=[rs_in[s].opt()], outs=[rs_out[s].opt()])

                    # final residual + output
                    for s in range(2):
                        rs_sb = fc2p.tile([128, D], BF16, tag="rssb", bufs=2,
                                          name=f"rssb{s}")
                        nc.sync.dma_start(rs_sb, rs_out[s])
                        o_sb = fc2p.tile([128, D], F32, tag="osb", bufs=2,
                                         name=f"osb{s}")
                        nc.vector.tensor_tensor(o_sb, rs_sb, x1[:, s, :], op=AL.add)
                        nc.sync.dma_start(out_io[128 * s:128 * s + 128, :], o_sb)

    nc.compile()
    return nc


def _wquant(w):
    """BitNet ternary weight quant; returns (int weights fp32, clip-mean)."""
    m = np.maximum(np.mean(np.abs(w), dtype=np.float32), np.float32(1e-5))
    ws = np.float32(1.0) / m
    wi = np.clip(np.round(w * ws), -1.0, 1.0).astype(np.float32)
    return wi, m


def kernel(x, g1, b1, g2, b2, wq, wk, wv, wo, w1, w2):
    x = np.asarray(x, np.float32)
    B, T_, D_ = x.shape
    assert (B, T_, D_) == (1, T, D)
    g1 = np.asarray(g1, np.float32); b1 = np.asarray(b1, np.float32)
    g2 = np.asarray(g2, np.float32); b2 = np.asarray(b2, np.float32)

    wq_i, mq = _wquant(np.asarray(wq, np.float32))
    wk_i, mk = _wquant(np.asarray(wk, np.float32))
    wv_i, mv = _wquant(np.asarray(wv, np.float32))
    wo_i, mo = _wquant(np.asarray(wo, np.float32))
    w1_i, m1 = _wquant(np.asarray(w1, np.float32))
    w2_i, m2 = _wquant(np.asarray(w2, np.float32))

    affine1 = not (np.all(g1 == 1.0) and np.all(b1 == 0.0))
    affine2 = not (np.all(g2 == 1.0) and np.all(b2 == 0.0))

    key = (affine1, affine2)
    if key not in _CACHE:
        _CACHE[key] = _build(affine1, affine2)
    nc = _CACHE[key]

    bf = ml_dtypes.bfloat16
    consts = np.zeros((1, 8), np.float32)
    consts[0, 0] = np.float32(0.125) * mq * mk
    consts[0, 1] = mv
    consts[0, 2] = mo
    consts[0, 3] = m1
    consts[0, 4] = m2
    cmask = np.triu(np.ones((128, 128), np.float32)).astype(bf)
    woT = np.ascontiguousarray(wo_i.T).astype(bf)
    w2TT = np.ascontiguousarray(w2_i.T)  # [8192, 2048]

    in_maps = []
    for c in range(NCORES):
        m = {
            "x": np.ascontiguousarray(x[0, TLOC * c:TLOC * (c + 1), :]),
            "wqT": np.ascontiguousarray(wq_i[QD * c:QD * (c + 1), :].T).astype(bf),
            "wkT": np.ascontiguousarray(np.concatenate(
                [wk_i[HD * c:HD * (c + 1), :].T] * 2, axis=1)).astype(bf),
            "wvT": np.ascontiguousarray(wv_i[HD * c:HD * (c + 1), :].T).astype(bf),
            "woT": woT,
            "w1T": np.ascontiguousarray(w1_i[FLOC * c:FLOC * (c + 1), :].T).astype(bf),
            "w2T": np.ascontiguousarray(w2TT[FLOC * c:FLOC * (c + 1), :]).astype(bf),
            "cmask": cmask,
            "consts": consts,
        }
        if affine1:
            m["g1"] = g1.reshape(1, D); m["b1"] = b1.reshape(1, D)
        if affine2:
            m["g2"] = g2.reshape(1, D); m["b2"] = b2.reshape(1, D)
        in_maps.append(m)

    trace = False
    if os.environ.get("KERNEL_TRACE") == "1":
        trace = _install_ntff_hook()
    r = bass_utils.run_bass_kernel_spmd(nc, in_maps, core_ids=list(range(NCORES)),
                                        trace=trace)
    global LAST_RESULT
    LAST_RESULT = r
    out = np.empty((T, D), np.float32)
    for c in range(NCORES):
        out[TLOC * c:TLOC * (c + 1), :] = r.results[c]["out"]
    return out.reshape(1, T, D)


LAST_RESULT = None


def _install_ntff_hook():
    """Recreate the antenv.axon_hooks module missing from this image so
    run_bass_kernel_spmd(trace=True) can capture NTFF profiles."""
    import sys, types
    if 'antenv.axon_hooks' not in sys.modules:
        mod = types.ModuleType('antenv.axon_hooks')
        mod._hook = None
        mod.set_axon_ntff_profile_hook = lambda h: setattr(mod, '_hook', h)
        mod.get_axon_ntff_profile_hook = lambda: mod._hook
        sys.modules['antenv.axon_hooks'] = mod
        import antenv
        antenv.axon_hooks = mod
    mod = sys.modules['antenv.axon_hooks']
    if mod.get_axon_ntff_profile_hook() is None:
        try:
            from trn_agent_boot.trn_boot import _ntff_profile_via_ctypes
            mod.set_axon_ntff_profile_hook(
                _ntff_profile_via_ctypes('/opt/axon/libaxon_pjrt.so'))
        except Exception:
            return False
    return mod.get_axon_ntff_profile_hook() is not None
